# revision 3
# baseline (speedup 1.0000x reference)
"""Grouped GEMM (MoE routing) Trainium2 kernel.

Full inputs in, full output out. Tensor-parallel shard of the output N
dimension across 8 NeuronCores (each core: all 8192 tokens x a 512-column
slice of N). Matmul orientation: weights stationary ([128 K, 128 N] tiles
loaded into the PE array), tokens as the moving dimension -- token segments
need no 128-padding, so the tensor engine streams exactly T*K*NS MACs.
Consecutive matmuls over the token-pieces of a segment share one stationary
tile. Activations/weights stream HBM->SBUF in bf16; outputs drain as bf16
in [N-slice, token-window] strips. The host-known segment schedule is baked
into the instruction stream; the host transposes/reassembles the output.
"""

import os
import sys
from contextlib import ExitStack

import numpy as np

for _p in ("/opt/trn_rl_repo", "/root/.axon_site/_ro/trn_rl_repo"):
    if os.path.isdir(_p) and _p not in sys.path:
        sys.path.insert(0, _p)

import concourse.bass as bass  # noqa: E402,F401
import concourse.tile as tile  # noqa: E402
from concourse import bacc, mybir  # noqa: E402

E, T, K, N = 8, 8192, 2048, 4096
NCORES = 8
NS = N // NCORES  # output columns per core (512)
P = 128
KB = K // P  # contraction blocks (16)
BT = 512  # tokens per activation block (moving-dim granularity)
NBLK = T // BT  # 16 activation blocks
WIN = 1024  # output window (tokens) per drain strip
NWIN = T // WIN
NSL = NS // P  # 4 stationary N-slices per core
LAST_RESULT = None  # BassKernelResults of the most recent run (for test.py)


def _dtypes():
    kind = os.environ.get("KERNEL_DTYPE", "bf16")
    if kind == "bf16":
        import ml_dtypes

        return mybir.dt.bfloat16, ml_dtypes.bfloat16
    if kind == "f32":
        return mybir.dt.float32, np.float32
    return mybir.dt.float32r, np.float32


def _schedule(seg):
    """Token-exact schedule. Returns (ss_list, window_last).

    ss_list: supersegments (slot, [(block, o0, o1, t0), ...]) with <=4
    pieces each; a piece is a segment's token run within one 512 block.
    window_last[w]: index in ss_list of the last supersegment touching
    output window w.
    """
    ss_list = []
    for s in range(E):
        lo, hi = int(seg[s]), int(seg[s + 1])
        if hi <= lo:
            continue
        pieces = []
        t = lo
        while t < hi:
            b = t // BT
            t1 = min((b + 1) * BT, hi)
            pieces.append((b, t - b * BT, t1 - b * BT, t))
            t = t1
        groups = [pieces[i : i + 4] for i in range(0, len(pieces), 4)]
        if len(groups) >= 2 and len(groups[-2]) > 1:
            # avoid a tiny trailing group (LDWEIGHTS-bound matmuls)
            if sum(p[2] - p[1] for p in groups[-1]) < 256:
                groups[-1].insert(0, groups[-2].pop())
        for g in groups:
            ss_list.append((s, g))
    window_last = {}
    for i, (_, pieces) in enumerate(ss_list):
        for b, _, _, t0 in pieces:
            window_last[t0 // WIN] = i
    return ss_list, window_last


def _build(seg_vals, mm_dt):
    nc = bacc.Bacc(
        "TRN2",
        target_bir_lowering=False,
        debug=False,
        enable_asserts=False,
        num_devices=NCORES,
    )
    f32 = mybir.dt.float32
    # SBUF-native HBM layouts (contiguous runs per partition line):
    # at[b, p, kb, t] = a[b*512 + t, kb*128 + p]
    # bt[s, p, kb, n] = b[widx[s], n_off + n, kb*128 + p]
    # out[nsl, p, t]  = c[t, n_off + nsl*128 + p]
    at = nc.dram_tensor("at", [NBLK, P, KB, BT], mm_dt, kind="ExternalInput").ap()
    bt = nc.dram_tensor("bt", [E, P, KB, NS], mm_dt, kind="ExternalInput").ap()
    out = nc.dram_tensor("out", [NSL, P, T], mm_dt, kind="ExternalOutput").ap()

    ss_list, window_last = _schedule(seg_vals)
    first_slot = ss_list[0][0]
    first_blocks = {b for b, _, _, _ in ss_list[0][1]}

    with tile.TileContext(nc) as tc, ExitStack() as ctx:
        # fine-grained startup tiles (2-kb slices) for the first-needed
        # weight slot + activation blocks; 4-kb slices elsewhere
        w0pool = ctx.enter_context(tc.tile_pool(name="w0", bufs=8))
        a0pool = ctx.enter_context(tc.tile_pool(name="a0", bufs=16))
        wpool = ctx.enter_context(tc.tile_pool(name="w", bufs=12))
        apool = ctx.enter_context(tc.tile_pool(name="a", bufs=16))
        pspool = ctx.enter_context(tc.tile_pool(name="ps", bufs=8, space="PSUM"))
        opool = ctx.enter_context(tc.tile_pool(name="o", bufs=12))

        # ---- DMA issue order: by first use --------------------------------
        first_use_a = {}
        first_use_w = {}
        for i, (s, pieces) in enumerate(ss_list):
            t_start = pieces[0][3]
            first_use_w.setdefault(s, t_start)
            for b, _, _, _ in pieces:
                first_use_a.setdefault(b, t_start)
        events = []  # (pos, sub-slice index, kind-priority, kind, idx, klo, khi)
        for s, pos in first_use_w.items():
            kg = 2 if (s == first_slot) else 4
            for g in range(KB // kg):
                events.append((pos, g * kg, 0, "w", s, g * kg, (g + 1) * kg))
        for b, pos in first_use_a.items():
            kg = 2 if (b in first_blocks) else 4
            for g in range(KB // kg):
                events.append((pos, g * kg, 1, "a", b, g * kg, (g + 1) * kg))
        events.sort(key=lambda e: (e[0], e[1], e[2], e[4]))

        # kb -> (tile, local kb index) maps
        a_sub = {b: [None] * KB for b in range(NBLK)}
        w_sub = {s: [None] * KB for s in first_use_w}
        for pos, _, _, kind, idx, klo, khi in events:
            kw = khi - klo
            if kind == "w":
                pool = w0pool if idx == first_slot else wpool
                st = pool.tile([P, kw, NS], mm_dt, tag="w", name="wt")
                nc.sync.dma_start(out=st[:], in_=bt[idx][:, klo:khi, :])
                for kb in range(klo, khi):
                    w_sub[idx][kb] = (st, kb - klo)
            else:
                pool = a0pool if idx in first_blocks else apool
                st = pool.tile([P, kw, BT], mm_dt, tag="a", name="atile")
                nc.sync.dma_start(out=st[:], in_=at[idx][:, klo:khi, :])
                for kb in range(klo, khi):
                    a_sub[idx][kb] = (st, kb - klo)

        # ---- compute + drain ----------------------------------------------
        strips = {}  # (w, nsl) -> strip tile

        def do_copies(ss_i, nsl, ps_tiles, pieces):
            for ps, (b, o0, o1, t0) in zip(ps_tiles, pieces):
                ln = o1 - o0
                w = t0 // WIN
                key = (w, nsl)
                if key not in strips:
                    strips[key] = opool.tile([P, WIN], mm_dt, tag="o", name="ostrip")
                c0 = t0 - w * WIN
                nc.vector.tensor_copy(strips[key][:, c0 : c0 + ln], ps[:, :ln])
            # flush any window this (ss, nsl) finishes
            for w in sorted({t0 // WIN for _, _, _, t0 in pieces}):
                if window_last[w] == ss_i:
                    nc.sync.dma_start(
                        out=out[nsl][:, w * WIN : (w + 1) * WIN],
                        in_=strips[(w, nsl)][:],
                    )

        for ss_i, (slot, pieces) in enumerate(ss_list):
            kb_outer = ss_i == 0 and len(pieces) <= 2
            if kb_outer:
                ps_all = [
                    [pspool.tile([P, BT], f32, tag="ps", name="ps") for _ in pieces]
                    for _ in range(NSL)
                ]
                for kb in range(KB):
                    for nsl in range(NSL):
                        wt, kl = w_sub[slot][kb]
                        lhsT = wt[:, kl, nsl * P : (nsl + 1) * P]
                        for j, (b, o0, o1, t0) in enumerate(pieces):
                            atile, akl = a_sub[b][kb]
                            nc.tensor.matmul(
                                ps_all[nsl][j][:, : o1 - o0],
                                lhsT=lhsT,
                                rhs=atile[:, akl, o0:o1],
                                start=(kb == 0),
                                stop=(kb == KB - 1),
                            )
                for nsl in range(NSL):
                    do_copies(ss_i, nsl, ps_all[nsl], pieces)
            else:
                for nsl in range(NSL):
                    ps_tiles = [pspool.tile([P, BT], f32, tag="ps", name="ps") for _ in pieces]
                    for kb in range(KB):
                        wt, kl = w_sub[slot][kb]
                        lhsT = wt[:, kl, nsl * P : (nsl + 1) * P]
                        for j, (b, o0, o1, t0) in enumerate(pieces):
                            atile, akl = a_sub[b][kb]
                            nc.tensor.matmul(
                                ps_tiles[j][:, : o1 - o0],
                                lhsT=lhsT,
                                rhs=atile[:, akl, o0:o1],
                                start=(kb == 0),
                                stop=(kb == KB - 1),
                            )
                    do_copies(ss_i, nsl, ps_tiles, pieces)

    nc.compile()
    return nc


def kernel(a, b, c, batch_size, weight_column_major, seg_indptr, weight_indices, **_):
    from concourse.bass_utils import run_bass_kernel_spmd

    global LAST_RESULT
    mm_dt, np_dt = _dtypes()

    a = np.asarray(a, dtype=np.float32)
    b = np.asarray(b, dtype=np.float32)
    seg = [int(x) for x in np.asarray(seg_indptr)]
    widx = [int(x) for x in np.asarray(weight_indices)]

    # at[b, p, kb, t] = a[b*512 + t, kb*128 + p]
    aT = np.ascontiguousarray(a.T).astype(np_dt, copy=False)  # [K, T]
    at_tiled = np.ascontiguousarray(
        aT.reshape(KB, P, NBLK, BT).transpose(2, 1, 0, 3)
    )  # [NBLK, P, KB, BT]

    bperm = b[widx]  # [E, N, K] in segment-slot order
    in_maps = []
    for cidx in range(NCORES):
        btc = np.swapaxes(bperm[:, cidx * NS : (cidx + 1) * NS, :], 1, 2)  # [E,K,NS]
        bt_tiled = np.ascontiguousarray(
            btc.reshape(E, KB, P, NS).transpose(0, 2, 1, 3)
        ).astype(np_dt, copy=False)  # [E, P, KB, NS]
        in_maps.append({"at": at_tiled, "bt": bt_tiled})

    nc = _build(seg, mm_dt)
    trace = bool(int(os.environ.get("KERNEL_TRACE", "0")))
    tmpdir = None
    if trace:
        import shutil

        tmpdir = os.environ.get("KERNEL_TRACE_DIR", "/tmp/ntff_out")
        shutil.rmtree(tmpdir, ignore_errors=True)
        os.makedirs(tmpdir, exist_ok=True)
    res = run_bass_kernel_spmd(
        nc,
        in_maps,
        core_ids=list(range(NCORES)),
        trace=trace,
        tmpdir=tmpdir,
    )
    LAST_RESULT = res

    # out[nsl, p, t] per core -> full [T, N] fp32
    full = np.empty((N, T), dtype=np.float32)
    for cidx in range(NCORES):
        oc = np.asarray(res.results[cidx]["out"]).reshape(NS, T)
        full[cidx * NS : (cidx + 1) * NS, :] = oc.astype(np.float32)
    return np.ascontiguousarray(full.T)


# revision 6
# speedup vs baseline: 1.0346x; 1.0346x over previous
"""Grouped GEMM (MoE routing) Trainium2 kernel.

Full inputs in, full output out. Tensor-parallel shard of the output N
dimension across 8 NeuronCores (each core: all 8192 tokens x a 512-column
slice of N). Matmul orientation: weights stationary ([128 K, 128 N] tiles
loaded into the PE array), tokens as the moving dimension -- token segments
need no 128-padding, so the tensor engine streams exactly T*K*NS MACs.
Input DMA triggers are emitted just-in-time (interleaved with compute) on
the Sync engine; output strips drain on the Scalar engine so the in-order
trigger queues never head-of-line block each other. Activations/weights
stream HBM->SBUF in bf16; outputs drain as bf16 [N-slice, 2048-token
window] strips. The host-known segment schedule is baked into the
instruction stream; the host transposes/reassembles the output.
"""

import os
import sys
from contextlib import ExitStack

import numpy as np

for _p in ("/opt/trn_rl_repo", "/root/.axon_site/_ro/trn_rl_repo"):
    if os.path.isdir(_p) and _p not in sys.path:
        sys.path.insert(0, _p)

import concourse.bass as bass  # noqa: E402,F401
import concourse.tile as tile  # noqa: E402
from concourse import bacc, mybir  # noqa: E402

E, T, K, N = 8, 8192, 2048, 4096
NCORES = 8
NS = N // NCORES  # output columns per core (512)
P = 128
KB = K // P  # contraction blocks (16)
BT = 512  # tokens per activation block (moving-dim granularity)
NBLK = T // BT  # 16 activation blocks
WIN = 2048  # output window (tokens) per drain strip
NSL = NS // P  # 4 stationary N-slices per core
PREFETCH_TOK = 800  # issue input DMA triggers this many tokens ahead
LAST_RESULT = None  # BassKernelResults of the most recent run (for test.py)


def _dtypes():
    kind = os.environ.get("KERNEL_DTYPE", "bf16")
    if kind == "bf16":
        import ml_dtypes

        return mybir.dt.bfloat16, ml_dtypes.bfloat16
    if kind == "f32":
        return mybir.dt.float32, np.float32
    return mybir.dt.float32r, np.float32


def _schedule(seg):
    """Token-exact schedule. Returns (ss_list, window_last).

    ss_list: supersegments (slot, [(block, o0, o1, t0), ...]) with <=4
    pieces each; a piece is a segment's token run within one 512 block.
    window_last[w]: index in ss_list of the last supersegment touching
    output window w.
    """
    ss_list = []
    for s in range(E):
        lo, hi = int(seg[s]), int(seg[s + 1])
        if hi <= lo:
            continue
        pieces = []
        t = lo
        while t < hi:
            b = t // BT
            t1 = min((b + 1) * BT, hi)
            pieces.append((b, t - b * BT, t1 - b * BT, t))
            t = t1
        groups = [pieces[i : i + 4] for i in range(0, len(pieces), 4)]
        if len(groups) >= 2 and len(groups[-2]) > 1:
            # avoid a tiny trailing group (LDWEIGHTS-bound matmuls)
            if sum(p[2] - p[1] for p in groups[-1]) < 256:
                groups[-1].insert(0, groups[-2].pop())
        for g in groups:
            ss_list.append((s, g))
    window_last = {}
    for i, (_, pieces) in enumerate(ss_list):
        for b, _, _, t0 in pieces:
            window_last[t0 // WIN] = i
    return ss_list, window_last


def _build(seg_vals, mm_dt):
    nc = bacc.Bacc(
        "TRN2",
        target_bir_lowering=False,
        debug=False,
        enable_asserts=False,
        num_devices=NCORES,
    )
    f32 = mybir.dt.float32
    # SBUF-native HBM layouts (contiguous runs per partition line):
    # at[b, p, kb, t] = a[b*512 + t, kb*128 + p]
    # bt[s, p, kb, n] = b[widx[s], n_off + n, kb*128 + p]
    # out[nsl, p, t]  = c[t, n_off + nsl*128 + p]
    at = nc.dram_tensor("at", [NBLK, P, KB, BT], mm_dt, kind="ExternalInput").ap()
    bt = nc.dram_tensor("bt", [E, P, KB, NS], mm_dt, kind="ExternalInput").ap()
    out = nc.dram_tensor("out", [NSL, P, T], mm_dt, kind="ExternalOutput").ap()

    ss_list, window_last = _schedule(seg_vals)
    ss_start = [pieces[0][3] for _, pieces in ss_list]
    first_slot = ss_list[0][0]
    first_blocks = {b for b, _, _, _ in ss_list[0][1]}

    with tile.TileContext(nc) as tc, ExitStack() as ctx:
        # fine-grained startup tiles for the first-needed weight slot +
        # activation blocks; whole-tile DMAs elsewhere
        w0pool = ctx.enter_context(tc.tile_pool(name="w0", bufs=5))
        a0pool = ctx.enter_context(tc.tile_pool(name="a0", bufs=10))
        wpool = ctx.enter_context(tc.tile_pool(name="w", bufs=3))
        apool = ctx.enter_context(tc.tile_pool(name="a", bufs=4))
        pspool = ctx.enter_context(tc.tile_pool(name="ps", bufs=8, space="PSUM"))
        opool = ctx.enter_context(tc.tile_pool(name="o", bufs=8))

        # ---- input DMA events, ordered by first use -----------------------
        first_use_a = {}
        first_use_w = {}
        for s, pieces in ss_list:
            t_start = pieces[0][3]
            first_use_w.setdefault(s, t_start)
            for b, _, _, _ in pieces:
                first_use_a.setdefault(b, t_start)
        events = []  # (pos, kb_lo, priority, kind, idx, klo, khi)
        for s, pos in first_use_w.items():
            granules = (2, 2, 4, 4, 4) if s == first_slot else (KB,)
            klo = 0
            for kg in granules:
                events.append((pos, klo, 0, "w", s, klo, klo + kg))
                klo += kg
        for b, pos in first_use_a.items():
            granules = (2, 2, 4, 4, 4) if b in first_blocks else (KB,)
            klo = 0
            for kg in granules:
                events.append((pos, klo, 1, "a", b, klo, klo + kg))
                klo += kg
        events.sort(key=lambda e: (e[0], e[1], e[2], e[4]))
        # assign each event the superseg index before which it is emitted:
        # the last superseg starting at-or-before the issue point
        import bisect

        ev_issue = []
        for ev in events:
            issue_tok = ev[0] - PREFETCH_TOK
            ev_issue.append(max(0, bisect.bisect_right(ss_start, issue_tok) - 1))

        a_sub = {b: [None] * KB for b in range(NBLK)}
        w_sub = {s: [None] * KB for s in first_use_w}

        def emit_input_dmas(ss_i):
            while events and ev_issue[0] <= ss_i:
                _, _, _, kind, idx, klo, khi = events.pop(0)
                ev_issue.pop(0)
                kw = khi - klo
                if kind == "w":
                    pool = w0pool if idx == first_slot else wpool
                    st = pool.tile([P, kw, NS], mm_dt, tag="w", name="wt")
                    nc.sync.dma_start(out=st[:], in_=bt[idx][:, klo:khi, :])
                    for kb in range(klo, khi):
                        w_sub[idx][kb] = (st, kb - klo)
                else:
                    pool = a0pool if idx in first_blocks else apool
                    st = pool.tile([P, kw, BT], mm_dt, tag="a", name="atile")
                    nc.sync.dma_start(out=st[:], in_=at[idx][:, klo:khi, :])
                    for kb in range(klo, khi):
                        a_sub[idx][kb] = (st, kb - klo)

        # ---- compute + drain ----------------------------------------------
        strips = {}  # (w, nsl) -> strip tile

        def do_copies(ss_i, nsl, ps_tiles, pieces):
            for ps, (b, o0, o1, t0) in zip(ps_tiles, pieces):
                ln = o1 - o0
                w = t0 // WIN
                key = (w, nsl)
                if key not in strips:
                    strips[key] = opool.tile([P, WIN], mm_dt, tag="o", name="ostrip")
                c0 = t0 - w * WIN
                nc.vector.tensor_copy(strips[key][:, c0 : c0 + ln], ps[:, :ln])
            # flush any window this (ss, nsl) finishes (Scalar-engine trigger)
            for w in sorted({t0 // WIN for _, _, _, t0 in pieces}):
                if window_last[w] == ss_i:
                    nc.scalar.dma_start(
                        out=out[nsl][:, w * WIN : (w + 1) * WIN],
                        in_=strips[(w, nsl)][:],
                    )

        for ss_i, (slot, pieces) in enumerate(ss_list):
            emit_input_dmas(ss_i)
            kb_outer = ss_i == 0 and len(pieces) <= 2
            if kb_outer:
                ps_all = [
                    [pspool.tile([P, BT], f32, tag="ps", name="ps") for _ in pieces]
                    for _ in range(NSL)
                ]
                for kb in range(KB):
                    for nsl in range(NSL):
                        wt, kl = w_sub[slot][kb]
                        lhsT = wt[:, kl, nsl * P : (nsl + 1) * P]
                        for j, (b, o0, o1, t0) in enumerate(pieces):
                            atile, akl = a_sub[b][kb]
                            nc.tensor.matmul(
                                ps_all[nsl][j][:, : o1 - o0],
                                lhsT=lhsT,
                                rhs=atile[:, akl, o0:o1],
                                start=(kb == 0),
                                stop=(kb == KB - 1),
                            )
                for nsl in range(NSL):
                    do_copies(ss_i, nsl, ps_all[nsl], pieces)
            else:
                for nsl in range(NSL):
                    ps_tiles = [
                        pspool.tile([P, BT], f32, tag="ps", name="ps") for _ in pieces
                    ]
                    for kb in range(KB):
                        wt, kl = w_sub[slot][kb]
                        lhsT = wt[:, kl, nsl * P : (nsl + 1) * P]
                        for j, (b, o0, o1, t0) in enumerate(pieces):
                            atile, akl = a_sub[b][kb]
                            nc.tensor.matmul(
                                ps_tiles[j][:, : o1 - o0],
                                lhsT=lhsT,
                                rhs=atile[:, akl, o0:o1],
                                start=(kb == 0),
                                stop=(kb == KB - 1),
                            )
                    do_copies(ss_i, nsl, ps_tiles, pieces)

    nc.compile()
    return nc


def kernel(a, b, c, batch_size, weight_column_major, seg_indptr, weight_indices, **_):
    from concourse.bass_utils import run_bass_kernel_spmd

    global LAST_RESULT
    mm_dt, np_dt = _dtypes()

    a = np.asarray(a, dtype=np.float32)
    b = np.asarray(b, dtype=np.float32)
    seg = [int(x) for x in np.asarray(seg_indptr)]
    widx = [int(x) for x in np.asarray(weight_indices)]

    # at[b, p, kb, t] = a[b*512 + t, kb*128 + p]
    aT = np.ascontiguousarray(a.T).astype(np_dt, copy=False)  # [K, T]
    at_tiled = np.ascontiguousarray(
        aT.reshape(KB, P, NBLK, BT).transpose(2, 1, 0, 3)
    )  # [NBLK, P, KB, BT]

    bperm = b[widx]  # [E, N, K] in segment-slot order
    in_maps = []
    for cidx in range(NCORES):
        btc = np.swapaxes(bperm[:, cidx * NS : (cidx + 1) * NS, :], 1, 2)  # [E,K,NS]
        bt_tiled = np.ascontiguousarray(
            btc.reshape(E, KB, P, NS).transpose(0, 2, 1, 3)
        ).astype(np_dt, copy=False)  # [E, P, KB, NS]
        in_maps.append({"at": at_tiled, "bt": bt_tiled})

    nc = _build(seg, mm_dt)
    trace = bool(int(os.environ.get("KERNEL_TRACE", "0")))
    tmpdir = None
    if trace:
        import shutil

        tmpdir = os.environ.get("KERNEL_TRACE_DIR", "/tmp/ntff_out")
        shutil.rmtree(tmpdir, ignore_errors=True)
        os.makedirs(tmpdir, exist_ok=True)
    res = run_bass_kernel_spmd(
        nc,
        in_maps,
        core_ids=list(range(NCORES)),
        trace=trace,
        tmpdir=tmpdir,
    )
    LAST_RESULT = res

    # out[nsl, p, t] per core -> full [T, N] fp32
    full = np.empty((N, T), dtype=np.float32)
    for cidx in range(NCORES):
        oc = np.asarray(res.results[cidx]["out"]).reshape(NS, T)
        full[cidx * NS : (cidx + 1) * NS, :] = oc.astype(np.float32)
    return np.ascontiguousarray(full.T)
